# revision 5
# baseline (speedup 1.0000x reference)
"""MoE ResNet18 (4 experts, top-1 gating) on 8 Trainium2 cores.

Strategy: top-1 gating means each of the 8 samples passes through exactly
one expert.  Gating/routing runs on host (it is the dispatch); each core
runs one sample through its routed expert's ResNet18.  Convs are emitted
as accumulated PE matmuls over shifted access patterns (channels on
partitions, spatial on the free dim), bf16 inputs with fp32 PSUM
accumulation.  BN (eval-mode scale/shift) is folded into the weights and
the ScalarE activation bias.  Output is scaled by the gate value on host.
"""

import sys

if "/opt/trn_rl_repo" not in sys.path:
    sys.path.insert(0, "/opt/trn_rl_repo")

import numpy as np
import ml_dtypes

E, NB, NCLS = 4, 8, 14
IMG = 384
BLOCK_SPECS = [(64, 64, 1), (64, 64, 1), (64, 128, 2), (128, 128, 1),
               (128, 256, 2), (256, 256, 1), (256, 512, 2), (512, 512, 1)]

DT_NP = ml_dtypes.bfloat16

# stage geometry: stage s feeds layer s blocks; stage0 = maxpool output
# (C, H) for the activation that block i consumes / produces
def block_geom(i):
    ci, co, st = BLOCK_SPECS[i]
    hin = 96 // (2 ** (i // 2)) * (1 if st == 1 else 2) // 1
    # derive from running H: blocks 0,1: 96; 2,3: 48; 4,5: 24; 6,7: 12 (output H)
    hout = 96 // (2 ** ((i // 2)))
    if st == 2:
        hin = hout * 2
    else:
        hin = hout
    return ci, co, st, hin, hout


def row_chunks(hout, wout, maxn=512):
    nr = max(1, min(hout, maxn // wout))
    return [(r, min(nr, hout - r)) for r in range(0, hout, nr)]


def cp_of(c):
    return min(128, c)


def nch_of(c):
    return (c + 127) // 128 if c > 128 else 1


# ---------------------------------------------------------------------------
# host-side weight preparation
# ---------------------------------------------------------------------------

def _prep_conv_w(w, s):
    """w [co, ci, kh, kw] fp32, s [co] -> [nchin, cin_p, KK, co] DT_NP."""
    w = np.asarray(w, np.float32) * np.asarray(s, np.float32)[:, None, None, None]
    co, ci, kh, kw = w.shape
    wt = w.transpose(1, 2, 3, 0).reshape(ci, kh * kw, co)
    cin_p = cp_of(ci)
    nchin = ci // cin_p
    return np.ascontiguousarray(
        wt.reshape(nchin, cin_p, kh * kw, co).astype(DT_NP))


def _prep_bias(b, cout):
    b = np.asarray(b, np.float32)
    cop = cp_of(cout)
    return np.ascontiguousarray(b.reshape(nch_of(cout), cop))


def prep_expert(params, e):
    """Extract expert e's arrays in device layout. Returns dict name->np."""
    out = {}
    sw = np.asarray(params["stem_w"], np.float32)[e]      # [64,3,7,7]
    ss = np.asarray(params["stem_s"], np.float32)[e]
    sb = np.asarray(params["stem_b"], np.float32)[e]
    w = sw * ss[:, None, None, None]
    wt = w.transpose(1, 2, 3, 0).reshape(147, 64)          # rows (c,ky,kx)
    wpad = np.zeros((160, 64), np.float32)
    wpad[:147] = wt
    out["w_stem"] = wpad.astype(DT_NP)
    out["b_stem"] = _prep_bias(sb, 64)
    for i, (ci, co, st) in enumerate(BLOCK_SPECS):
        bp = params["blocks"][i]
        g = lambda k: np.asarray(bp[k], np.float32)[e]
        out[f"b{i}w1"] = _prep_conv_w(g("w1"), g("s1"))
        out[f"b{i}w1_b"] = _prep_bias(g("b1"), co)
        out[f"b{i}w2"] = _prep_conv_w(g("w2"), g("s2"))
        combb = g("b2").copy()
        if "wd" in bp:
            out[f"b{i}wd"] = _prep_conv_w(g("wd"), g("sd"))
            combb = combb + g("bd")
        out[f"b{i}comb_b"] = _prep_bias(combb, co)
    out["w_fc"] = np.ascontiguousarray(
        np.asarray(params["fc_w"], np.float32)[e].reshape(4, 128, NCLS))
    out["b_fc"] = np.asarray(params["fc_b"], np.float32)[e].reshape(1, NCLS)
    return out


def im2col_stem(xb):
    """xb [3,384,384] fp32 -> [147, 36864] DT_NP  (rows ordered (c,ky,kx))."""
    xp = np.zeros((3, IMG + 6, IMG + 6), np.float32)
    xp[:, 3:3 + IMG, 3:3 + IMG] = xb
    v = np.lib.stride_tricks.sliding_window_view(xp, (7, 7), axis=(1, 2))
    v = v[:, ::2, ::2]                       # [3,192,192,7,7]
    v = v.transpose(0, 3, 4, 1, 2).reshape(147, 192 * 192)
    out = np.zeros((160, 192 * 192), DT_NP)
    out[:147] = v.astype(DT_NP)
    return out


def host_gating(x, w_gate):
    xf = np.asarray(x, np.float32).reshape(NB, -1)
    logits = xf @ np.asarray(w_gate, np.float32)
    m = logits.max(axis=1, keepdims=True)
    ex = np.exp(logits - m)
    probs = ex / ex.sum(axis=1, keepdims=True)
    idx = probs.argmax(axis=1)
    tv = probs[np.arange(NB), idx]
    g = tv / (tv + np.float32(1e-6))
    return idx.astype(np.int64), g.astype(np.float32)


# ---------------------------------------------------------------------------
# bass program
# ---------------------------------------------------------------------------

MAXW = 1


def split_waits(nc, mybir):
    cnt = 0
    for fn in nc.m.functions:
        for bb in fn.blocks:
            new = []
            for inst in bb.instructions:
                si = inst.sync_info
                if si is not None and si.on_wait is not None and len(si.on_wait) > MAXW:
                    waits = list(si.on_wait)
                    while len(waits) > MAXW:
                        chunk, waits = waits[:MAXW], waits[MAXW:]
                        cnt += 1
                        nop = mybir.InstNoOp(name=f"waitsplit_{cnt}", ins=[], outs=[])
                        nop.engine = inst.engine
                        nop.sync_info = mybir.SyncInfo(on_wait=chunk, on_update=[])
                        new.append(nop)
                    inst.sync_info = mybir.SyncInfo(
                        on_wait=waits, on_update=list(si.on_update))
                new.append(inst)
            bb.instructions = new
    return cnt


def build_nc(debug_taps=()):
    import concourse.bass as bass
    import concourse.mybir as mybir
    import concourse.tile as tile

    DT = mybir.dt.bfloat16
    F32 = mybir.dt.float32
    AF = mybir.ActivationFunctionType

    nc = bass.Bass()

    # ---- dram parameters ----
    imc = nc.declare_dram_parameter("imc", [160, 36864], DT, isOutput=False)
    w_stem = nc.declare_dram_parameter("w_stem", [160, 64], DT, isOutput=False)
    b_stem = nc.declare_dram_parameter("b_stem", [1, 64], F32, isOutput=False)
    wdram, bdram = {}, {}
    for i, (ci, co, st) in enumerate(BLOCK_SPECS):
        cin_p, nchin = cp_of(ci), nch_of(ci)
        cop, ncho = cp_of(co), nch_of(co)
        wdram[f"b{i}w1"] = nc.declare_dram_parameter(
            f"b{i}w1", [nchin, cin_p, 9, co], DT, isOutput=False)
        bdram[f"b{i}w1_b"] = nc.declare_dram_parameter(
            f"b{i}w1_b", [ncho, cop], F32, isOutput=False)
        wdram[f"b{i}w2"] = nc.declare_dram_parameter(
            f"b{i}w2", [ncho, cop, 9, co], DT, isOutput=False)
        if st != 1 or ci != co:
            wdram[f"b{i}wd"] = nc.declare_dram_parameter(
                f"b{i}wd", [nchin, cin_p, 1, co], DT, isOutput=False)
        bdram[f"b{i}comb_b"] = nc.declare_dram_parameter(
            f"b{i}comb_b", [ncho, cop], F32, isOutput=False)
    w_fc = nc.declare_dram_parameter("w_fc", [4, 128, NCLS], F32, isOutput=False)
    b_fc = nc.declare_dram_parameter("b_fc", [1, NCLS], F32, isOutput=False)
    out_d = nc.declare_dram_parameter("out", [NCLS, 1], F32, isOutput=True)

    dbg = {}
    # activation tile geometries per layer: (C, H)
    LGEOM = [(64, 96), (128, 48), (256, 24), (512, 12)]
    for t in debug_taps:
        lay = {"pool": 0, "b0": 0, "b1": 0, "b2": 1, "b3": 1,
               "b4": 2, "b5": 2, "b6": 3, "b7": 3}[t]
        C, H = LGEOM[lay]
        dbg[t] = nc.declare_dram_parameter(
            f"dbg_{t}", [cp_of(C), nch_of(C) * (H + 2) * (H + 2)], DT, isOutput=True)

    from contextlib import ExitStack
    with tile.TileContext(nc) as tc, ExitStack() as stack:
        acts = stack.enter_context(tc.tile_pool(name="acts", bufs=1))
        wp = stack.enter_context(tc.tile_pool(name="wp", bufs=1))
        psum = stack.enter_context(tc.tile_pool(name="psum", bufs=6, space="PSUM"))

        # ---- persistent activation tiles ----
        def act_tile(name, C, H):
            t = acts.tile([cp_of(C), nch_of(C), (H + 2) * (H + 2)], DT, tag=name)
            nc.vector.memset(t[:], 0.0)
            return t

        tiles = {}
        for li, (C, H) in enumerate(LGEOM):
            for nm in ("A", "T", "B"):
                tiles[(li, nm)] = act_tile(f"l{li}{nm}", C, H)

        # ---- persistent weights: stem, L1, L2 convs, all biases, fc ----
        ws1 = wp.tile([128, 64], DT, tag="ws1")
        ws2 = wp.tile([32, 64], DT, tag="ws2")
        nc.sync.dma_start(ws1[:], w_stem[0:128, :])
        nc.sync.dma_start(ws2[:], w_stem[128:160, :])
        bstem_sb = wp.tile([64, 1], F32, tag="bstem")
        nc.sync.dma_start(bstem_sb[:], b_stem.rearrange("c p -> p c"))

        bias_sb = {}
        for key, d in bdram.items():
            ncho, cop = d.shape
            t = wp.tile([cop, ncho], F32, tag=f"bias_{key}")
            nc.sync.dma_start(t[:], d.rearrange("c p -> p c"))
            bias_sb[key] = t

        wfc_sb = wp.tile([128, 4, NCLS], F32, tag="wfc")
        nc.sync.dma_start(wfc_sb[:], w_fc.rearrange("c p m -> p c m"))
        bfc_sb = wp.tile([NCLS, 1], F32, tag="bfc")
        nc.sync.dma_start(bfc_sb[:], b_fc.rearrange("c p -> p c"))

        # persistent conv weights for blocks 0..3 (L1, L2)
        wsb_persist = {}
        for i in range(4):
            for key in (f"b{i}w1", f"b{i}w2", f"b{i}wd"):
                if key not in wdram:
                    continue
                d = wdram[key]
                nchin, cin_p, KK, co = d.shape
                ts = []
                for ci in range(nchin):
                    t = wp.tile([cin_p, KK, co], DT, tag=f"w_{key}_{ci}")
                    nc.sync.dma_start(t[:], d[ci])
                    ts.append(t)
                wsb_persist[key] = ts

        # ---- stem + maxpool ----
        A1 = tiles[(0, "A")]
        with tc.tile_pool(name="stems", bufs=2) as stems, \
             tc.tile_pool(name="stemt", bufs=2) as stemt, \
             tc.tile_pool(name="imcp", bufs=4) as imcp:
            prev_strip = None
            for s in range(8):
                strip = stems.tile([64, 4800], DT, tag="strip")
                if s > 0:
                    nc.vector.tensor_copy(strip[:, 0:192], prev_strip[:, 4608:4800])
                c0 = s * 4608
                for k in range(9):
                    t128 = imcp.tile([128, 512], DT, tag="i128")
                    t32 = imcp.tile([32, 512], DT, tag="i32")
                    nc.sync.dma_start(t128[:], imc[0:128, c0 + k * 512:c0 + (k + 1) * 512])
                    nc.sync.dma_start(t32[:], imc[128:160, c0 + k * 512:c0 + (k + 1) * 512])
                    ps = psum.tile([128, 512], F32, tag="ps", name="ps_stem")[:64, :]
                    nc.tensor.matmul(ps, ws1[:], t128[:], start=True, stop=False)
                    nc.tensor.matmul(ps, ws2[:], t32[:], start=False, stop=True)
                    nc.scalar.activation(strip[:, 192 + k * 512:192 + (k + 1) * 512],
                                         ps, AF.Relu, bias=bstem_sb[:, 0:1])
                # maxpool rows: strip rows st3[h]: h=0 halo(row 24s-1), h=1+l = row 24s+l
                st3 = strip.rearrange("p (h w) -> p h w", w=192)
                t = stemt.tile([64, 12, 192], DT, tag="pt")
                A_ = st3[:, 1:25:2, :]
                B_ = st3[:, 2:25:2, :]
                nc.vector.tensor_max(t[:], A_, B_)
                if s == 0:
                    C0 = st3[:, 2:23:2, :]
                    nc.vector.tensor_max(t[:, 1:12, :], t[:, 1:12, :], C0)
                else:
                    C_ = st3[:, 0:23:2, :]
                    nc.vector.tensor_max(t[:], t[:], C_)
                Ev = t[:, :, 0:191:2]
                Ov = t[:, :, 1:192:2]
                dstv = A1.rearrange("p c (h w) -> p c h w", w=98)[:, 0]
                R0 = 12 * s + 1
                nc.vector.tensor_max(dstv[:, R0:R0 + 12, 1:97], Ev, Ov)
                nc.vector.tensor_max(dstv[:, R0:R0 + 12, 2:97],
                                     dstv[:, R0:R0 + 12, 2:97], Ov[:, :, 0:95])
                prev_strip = strip

        if "pool" in dbg:
            nc.sync.dma_start(dbg["pool"][:], A1.rearrange("p c f -> p (c f)"))

        # ---- conv emission ----
        wstream = stack.enter_context(tc.tile_pool(name="wstream", bufs=8))
        wsb_stream = {}

        def get_wsb(key):
            if key in wsb_persist:
                return wsb_persist[key]
            if key not in wsb_stream:
                d = wdram[key]
                nchin, cin_p, KK, co = d.shape
                ts = []
                for ci in range(nchin):
                    t = wstream.tile([cin_p, KK, co], DT, tag="wstream")
                    nc.sync.dma_start(t[:], d[ci])
                    ts.append(t)
                wsb_stream[key] = ts
            return wsb_stream[key]

        def conv_rhs(srcv, ci, hin, wout, stride, r0, nr, ky, kx):
            if stride == 1:
                return srcv[:, ci, r0 + ky:r0 + ky + nr, kx:kx + wout]
            return srcv[:, ci, 2 * r0 + ky:2 * r0 + ky + 2 * nr - 1:2,
                        kx:kx + 2 * wout - 1:2]

        def emit_conv(src, dst, i, wkey, mode, res_src=None):
            """mode: 'relu' (conv1) | 'res' (conv2+identity) | 'res_ds' (conv2+ds)."""
            ci_, co_, st_ = BLOCK_SPECS[i]
            if wkey.endswith("w2"):
                cin, cout, stride = co_, co_, 1
                hin = hout = 96 // (2 ** (i // 2))
                bias = bias_sb[f"b{i}comb_b"]
            else:
                cin, cout, stride = ci_, co_, st_
                hout = 96 // (2 ** (i // 2))
                hin = hout * stride
                bias = bias_sb[f"b{i}w1_b"]
            wout = hout
            cin_p, nchin = cp_of(cin), nch_of(cin)
            cop, ncho = cp_of(cout), nch_of(cout)
            wpin = hin + 2
            srcv = src.rearrange("p c (h w) -> p c h w", w=wpin)
            dstv = dst.rearrange("p c (h w) -> p c h w", w=hout + 2)
            wts = get_wsb(wkey)
            if mode == "res_ds":
                wdts = get_wsb(f"b{i}wd")
                nch_ds = nch_of(ci_)
                resv = res_src.rearrange("p c (h w) -> p c h w", w=hout * st_ + 2)
            if mode == "res":
                resv = res_src.rearrange("p c (h w) -> p c h w", w=hout + 2)
            offs = [(a, b) for a in range(3) for b in range(3)]
            for co in range(ncho):
                co_sl = slice(co * 128, co * 128 + cop)
                for (r0, nr) in row_chunks(hout, wout):
                    N = nr * wout
                    ps = psum.tile([128, 512], F32, tag="ps", name="ps_conv")[:cop, :N]
                    ps3 = ps.rearrange("p (a b) -> p a b", b=wout)
                    n_mm = nchin * 9 + (nch_ds if mode == "res_ds" else 0)
                    idx = 0
                    for ci in range(nchin):
                        for k, (ky, kx) in enumerate(offs):
                            rhs = conv_rhs(srcv, ci, hin, wout, stride, r0, nr, ky, kx)
                            nc.tensor.matmul(ps, wts[ci][:, k, co_sl], rhs,
                                             start=(idx == 0), stop=(idx == n_mm - 1))
                            idx += 1
                    if mode == "res_ds":
                        for ci in range(nch_ds):
                            rhs = resv[:, ci, 2 * r0 + 1:2 * r0 + 1 + 2 * nr - 1:2,
                                       1:1 + 2 * wout - 1:2]
                            nc.tensor.matmul(ps, wdts[ci][:, 0, co_sl], rhs,
                                             start=False, stop=(idx == n_mm - 1))
                            idx += 1
                    dchunk = dstv[:, co, r0 + 1:r0 + 1 + nr, 1:1 + wout]
                    if mode == "res":
                        rchunk = resv[:, co, r0 + 1:r0 + 1 + nr, 1:1 + wout]
                        nc.vector.tensor_add(dchunk, ps3, rchunk)
                        nc.scalar.activation(dchunk, dchunk, AF.Relu,
                                             bias=bias[:, co:co + 1])
                    else:
                        nc.scalar.activation(dchunk, ps3, AF.Relu,
                                             bias=bias[:, co:co + 1])

        # block sequence
        # L1: A->T->B (b0), B->T->A (b1)
        # L(k): prevF -> T -> B (first, ds), B -> T -> A(=F) (second)
        seq = []
        prevF = tiles[(0, "A")]
        for li in range(4):
            T, Bt = tiles[(li, "T")], tiles[(li, "B")]
            i0, i1 = 2 * li, 2 * li + 1
            if li == 0:
                A = tiles[(0, "A")]
                seq.append((i0, A, T, Bt, False))
                seq.append((i1, Bt, T, A, False))
                prevF = A
            else:
                F = tiles[(li, "A")]
                seq.append((i0, prevF, T, Bt, True))
                seq.append((i1, Bt, T, F, False))
                prevF = F

        for (i, src, T, dst, has_ds) in seq:
            emit_conv(src, T, i, f"b{i}w1", "relu")
            if has_ds:
                emit_conv(T, dst, i, f"b{i}w2", "res_ds", res_src=src)
            else:
                emit_conv(T, dst, i, f"b{i}w2", "res", res_src=src)
            tap = f"b{i}"
            if tap in dbg:
                nc.sync.dma_start(dbg[tap][:], dst.rearrange("p c f -> p (c f)"))

        # ---- global avg pool + fc (fp32) ----
        F4 = prevF
        f4v = F4.rearrange("p c (h w) -> p c h w", w=14)
        h = acts.tile([128, 4], F32, tag="h")
        for c in range(4):
            nc.vector.reduce_sum(h[:, c:c + 1], f4v[:, c, 1:13, 1:13],
                                 axis=mybir.AxisListType.XY)
        h1 = acts.tile([128, 4], F32, tag="h1")
        nc.scalar.activation(h1[:], h[:], AF.Copy, scale=float(1.0 / 144.0))
        pf = psum.tile([128, 512], F32, tag="ps", name="ps_fc")[:NCLS, :1]
        for c in range(4):
            nc.tensor.matmul(pf, wfc_sb[:, c, :], h1[:, c:c + 1],
                             start=(c == 0), stop=(c == 3))
        osb = acts.tile([NCLS, 1], F32, tag="osb")
        nc.scalar.activation(osb[:], pf, AF.Identity, bias=bfc_sb[:, 0:1])
        nc.sync.dma_start(out_d[:], osb[:])

    split_waits(nc, mybir)
    return nc


# ---------------------------------------------------------------------------
# entry point
# ---------------------------------------------------------------------------

_CACHE = {}


def _get_nc(debug_taps=()):
    key = ("nc", tuple(debug_taps))
    if key not in _CACHE:
        _CACHE[key] = build_nc(debug_taps)
    return _CACHE[key]


def run_cores(in_maps, debug_taps=(), trace=False, tmpdir=None):
    from concourse.bass_utils import run_bass_kernel_spmd
    nc = _get_nc(debug_taps)
    return run_bass_kernel_spmd(nc, in_maps, list(range(NB)), trace=trace,
                                tmpdir=tmpdir)


def kernel(x, w_gate, params):
    x = np.asarray(x, np.float32)
    idx, g = host_gating(x, w_gate)
    experts = {}
    for e in set(idx.tolist()):
        experts[e] = prep_expert(params, e)
    in_maps = []
    for b in range(NB):
        m = dict(experts[idx[b]])
        m["imc"] = im2col_stem(x[b])
        in_maps.append(m)
    res = run_cores(in_maps)
    out = np.stack([res.results[b]["out"][:, 0] for b in range(NB)], axis=0)
    return (out * g[:, None]).astype(np.float32)


# revision 7
# speedup vs baseline: 1.4848x; 1.4848x over previous
"""MoE ResNet18 (4 experts, top-1 gating) on 8 Trainium2 cores.

Strategy: top-1 gating means each of the 8 samples passes through exactly
one expert.  Gating/routing runs on host (it is the dispatch); each core
runs one sample through its routed expert's ResNet18.  Convs are emitted
as accumulated PE matmuls over shifted access patterns (channels on
partitions, spatial on the free dim), bf16 inputs with fp32 PSUM
accumulation.  BN (eval-mode scale/shift) is folded into the weights and
the ScalarE activation bias.  Output is scaled by the gate value on host.
"""

import sys

if "/opt/trn_rl_repo" not in sys.path:
    sys.path.insert(0, "/opt/trn_rl_repo")

import numpy as np
import ml_dtypes

E, NB, NCLS = 4, 8, 14
IMG = 384
BLOCK_SPECS = [(64, 64, 1), (64, 64, 1), (64, 128, 2), (128, 128, 1),
               (128, 256, 2), (256, 256, 1), (256, 512, 2), (512, 512, 1)]

DT_NP = ml_dtypes.bfloat16

# stage geometry: stage s feeds layer s blocks; stage0 = maxpool output
# (C, H) for the activation that block i consumes / produces
def block_geom(i):
    ci, co, st = BLOCK_SPECS[i]
    hin = 96 // (2 ** (i // 2)) * (1 if st == 1 else 2) // 1
    # derive from running H: blocks 0,1: 96; 2,3: 48; 4,5: 24; 6,7: 12 (output H)
    hout = 96 // (2 ** ((i // 2)))
    if st == 2:
        hin = hout * 2
    else:
        hin = hout
    return ci, co, st, hin, hout


def row_chunks(hout, wout, maxn=512):
    nr = max(1, min(hout, maxn // wout))
    return [(r, min(nr, hout - r)) for r in range(0, hout, nr)]


def cp_of(c):
    return min(128, c)


def nch_of(c):
    return (c + 127) // 128 if c > 128 else 1


# ---------------------------------------------------------------------------
# host-side weight preparation
# ---------------------------------------------------------------------------

def _prep_conv_w(w, s):
    """w [co, ci, kh, kw] fp32, s [co] -> [nchin, cin_p, KK, co] DT_NP."""
    w = np.asarray(w, np.float32) * np.asarray(s, np.float32)[:, None, None, None]
    co, ci, kh, kw = w.shape
    wt = w.transpose(1, 2, 3, 0).reshape(ci, kh * kw, co)
    cin_p = cp_of(ci)
    nchin = ci // cin_p
    return np.ascontiguousarray(
        wt.reshape(nchin, cin_p, kh * kw, co).astype(DT_NP))


PACKED = ("b0w1", "b0w2", "b1w1", "b1w2", "b2w1")


def _prep_conv_w_packed(w, s):
    """w [co, 64, 3, 3] -> [6, 128, co]: groups 0-2 = ky pairs (kx0|kx1 stacked
    on partition halves), groups 3-5 = kx=2 singles (duplicated halves)."""
    w = np.asarray(w, np.float32) * np.asarray(s, np.float32)[:, None, None, None]
    co, ci, kh, kw = w.shape
    assert ci == 64 and kh == 3 and kw == 3
    wt = w.transpose(1, 2, 3, 0)                     # [64, 3, 3, co]
    out = np.zeros((6, 128, co), np.float32)
    for ky in range(3):
        out[ky, 0:64] = wt[:, ky, 0, :]
        out[ky, 64:128] = wt[:, ky, 1, :]
        out[3 + ky, 0:64] = wt[:, ky, 2, :]
        out[3 + ky, 64:128] = wt[:, ky, 2, :]
    return np.ascontiguousarray(out.astype(DT_NP))


def _prep_bias(b, cout):
    b = np.asarray(b, np.float32)
    cop = cp_of(cout)
    return np.ascontiguousarray(b.reshape(nch_of(cout), cop))


def prep_expert(params, e):
    """Extract expert e's arrays in device layout. Returns dict name->np."""
    out = {}
    sw = np.asarray(params["stem_w"], np.float32)[e]      # [64,3,7,7]
    ss = np.asarray(params["stem_s"], np.float32)[e]
    sb = np.asarray(params["stem_b"], np.float32)[e]
    w = sw * ss[:, None, None, None]
    wt = w.transpose(1, 2, 3, 0).reshape(147, 64)          # rows (c,ky,kx)
    wpad = np.zeros((160, 64), np.float32)
    wpad[:147] = wt
    out["w_stem"] = wpad.astype(DT_NP)
    out["w_stem2t"] = np.ascontiguousarray(
        np.concatenate([wpad[128:160], wpad[128:160]], axis=0).astype(DT_NP))
    out["b_stem"] = _prep_bias(sb, 64)
    for i, (ci, co, st) in enumerate(BLOCK_SPECS):
        bp = params["blocks"][i]
        g = lambda k: np.asarray(bp[k], np.float32)[e]
        if f"b{i}w1" in PACKED:
            out[f"b{i}w1"] = _prep_conv_w_packed(g("w1"), g("s1"))
        else:
            out[f"b{i}w1"] = _prep_conv_w(g("w1"), g("s1"))
        out[f"b{i}w1_b"] = _prep_bias(g("b1"), co)
        if f"b{i}w2" in PACKED:
            out[f"b{i}w2"] = _prep_conv_w_packed(g("w2"), g("s2"))
        else:
            out[f"b{i}w2"] = _prep_conv_w(g("w2"), g("s2"))
        combb = g("b2").copy()
        if "wd" in bp:
            out[f"b{i}wd"] = _prep_conv_w(g("wd"), g("sd"))
            combb = combb + g("bd")
        out[f"b{i}comb_b"] = _prep_bias(combb, co)
    out["w_fc"] = np.ascontiguousarray(
        np.asarray(params["fc_w"], np.float32)[e].reshape(4, 128, NCLS))
    out["b_fc"] = np.asarray(params["fc_b"], np.float32)[e].reshape(1, NCLS)
    return out


def im2col_stem(xb):
    """xb [3,384,384] fp32 -> [147, 36864] DT_NP  (rows ordered (c,ky,kx))."""
    xp = np.zeros((3, IMG + 6, IMG + 6), np.float32)
    xp[:, 3:3 + IMG, 3:3 + IMG] = xb
    v = np.lib.stride_tricks.sliding_window_view(xp, (7, 7), axis=(1, 2))
    v = v[:, ::2, ::2]                       # [3,192,192,7,7]
    v = v.transpose(0, 3, 4, 1, 2).reshape(147, 192 * 192)
    out = np.zeros((160, 192 * 192), DT_NP)
    out[:147] = v.astype(DT_NP)
    return out


def host_gating(x, w_gate):
    xf = np.asarray(x, np.float32).reshape(NB, -1)
    logits = xf @ np.asarray(w_gate, np.float32)
    m = logits.max(axis=1, keepdims=True)
    ex = np.exp(logits - m)
    probs = ex / ex.sum(axis=1, keepdims=True)
    idx = probs.argmax(axis=1)
    tv = probs[np.arange(NB), idx]
    g = tv / (tv + np.float32(1e-6))
    return idx.astype(np.int64), g.astype(np.float32)


# ---------------------------------------------------------------------------
# bass program
# ---------------------------------------------------------------------------

MAXW = 1


def split_waits(nc, mybir):
    cnt = 0
    for fn in nc.m.functions:
        for bb in fn.blocks:
            new = []
            for inst in bb.instructions:
                si = inst.sync_info
                if si is not None and si.on_wait is not None and len(si.on_wait) > MAXW:
                    waits = list(si.on_wait)
                    while len(waits) > MAXW:
                        chunk, waits = waits[:MAXW], waits[MAXW:]
                        cnt += 1
                        nop = mybir.InstNoOp(name=f"waitsplit_{cnt}", ins=[], outs=[])
                        nop.engine = inst.engine
                        nop.sync_info = mybir.SyncInfo(on_wait=chunk, on_update=[])
                        new.append(nop)
                    inst.sync_info = mybir.SyncInfo(
                        on_wait=waits, on_update=list(si.on_update))
                new.append(inst)
            bb.instructions = new
    return cnt


def build_nc(debug_taps=()):
    import concourse.bass as bass
    import concourse.mybir as mybir
    import concourse.tile as tile
    from contextlib import ExitStack

    DT = mybir.dt.bfloat16
    F32 = mybir.dt.float32
    AF = mybir.ActivationFunctionType

    nc = bass.Bass()

    # ---- dram parameters ----
    imc = nc.declare_dram_parameter("imc", [160, 36864], DT, isOutput=False)
    w_stem = nc.declare_dram_parameter("w_stem", [160, 64], DT, isOutput=False)
    w_stem2t = nc.declare_dram_parameter("w_stem2t", [64, 64], DT, isOutput=False)
    b_stem = nc.declare_dram_parameter("b_stem", [1, 64], F32, isOutput=False)
    wdram, bdram = {}, {}
    for i, (ci, co, st) in enumerate(BLOCK_SPECS):
        cin_p, nchin = cp_of(ci), nch_of(ci)
        cop, ncho = cp_of(co), nch_of(co)
        for wk, cc, nn in ((f"b{i}w1", ci, (nchin, cin_p)), (f"b{i}w2", co, (ncho, cop))):
            if wk in PACKED:
                wdram[wk] = nc.declare_dram_parameter(wk, [6, 128, co], DT, isOutput=False)
            else:
                wdram[wk] = nc.declare_dram_parameter(wk, [nn[0], nn[1], 9, co], DT, isOutput=False)
        bdram[f"b{i}w1_b"] = nc.declare_dram_parameter(
            f"b{i}w1_b", [ncho, cop], F32, isOutput=False)
        if st != 1 or ci != co:
            wdram[f"b{i}wd"] = nc.declare_dram_parameter(
                f"b{i}wd", [nchin, cin_p, 1, co], DT, isOutput=False)
        bdram[f"b{i}comb_b"] = nc.declare_dram_parameter(
            f"b{i}comb_b", [ncho, cop], F32, isOutput=False)
    w_fc = nc.declare_dram_parameter("w_fc", [4, 128, NCLS], F32, isOutput=False)
    b_fc = nc.declare_dram_parameter("b_fc", [1, NCLS], F32, isOutput=False)
    out_d = nc.declare_dram_parameter("out", [NCLS, 1], F32, isOutput=True)

    dbg = {}
    LGEOM = [(64, 96), (128, 48), (256, 24), (512, 12)]
    for t in debug_taps:
        lay = {"pool": 0, "b0": 0, "b1": 0, "b2": 1, "b3": 1,
               "b4": 2, "b5": 2, "b6": 3, "b7": 3}[t]
        C, H = LGEOM[lay]
        dbg[t] = nc.declare_dram_parameter(
            f"dbg_{t}", [cp_of(C), nch_of(C) * (H + 2) * (H + 2)], DT, isOutput=True)

    with tile.TileContext(nc) as tc, ExitStack() as stack:
        acts = stack.enter_context(tc.tile_pool(name="acts", bufs=1))
        wp = stack.enter_context(tc.tile_pool(name="wp", bufs=1))
        psum = stack.enter_context(tc.tile_pool(name="psum", bufs=6, space="PSUM"))

        # ---- PE warmup: full-array matmuls to lift the HAM clock gate while
        # the first im2col DMAs are in flight ----
        warm = acts.tile([128, 640], DT, tag="warm")
        nc.vector.memset(warm[:], 0.0)
        for j in range(6):
            pw = psum.tile([128, 512], F32, tag="ps", name=f"warm{j}")
            for k in range(8):
                nc.tensor.matmul(pw[:], warm[:, 0:128], warm[:, 128:640],
                                 start=(k == 0), stop=(k == 7))

        # ---- persistent activation tiles ----
        # layer 0 tiles are [128, ...]: partitions 64-127 hold the +1-column
        # shifted duplicate used for offset-pair / col-pack PE packing.
        def act_tile(name, C, H, dup=False):
            p = 128 if dup else cp_of(C)
            t = acts.tile([p, nch_of(C), (H + 2) * (H + 2)], DT, tag=name)
            nc.vector.memset(t[:], 0.0)
            return t

        tiles = {}
        for li, (C, H) in enumerate(LGEOM):
            for nm in ("A", "T", "B"):
                tiles[(li, nm)] = act_tile(f"l{li}{nm}", C, H, dup=(li == 0))

        # ---- persistent small weights ----
        ws1 = wp.tile([128, 64], DT, tag="ws1")
        ws2t = wp.tile([64, 64], DT, tag="ws2t")
        nc.sync.dma_start(ws1[:], w_stem[0:128, :])
        nc.sync.dma_start(ws2t[:], w_stem2t[:])
        bstem_sb = wp.tile([64, 1], F32, tag="bstem")
        nc.sync.dma_start(bstem_sb[:], b_stem.rearrange("c p -> p c"))

        bias_sb = {}
        for key, d in bdram.items():
            ncho, cop = d.shape
            t = wp.tile([cop, ncho], F32, tag=f"bias_{key}")
            nc.sync.dma_start(t[:], d.rearrange("c p -> p c"))
            bias_sb[key] = t

        wfc_sb = wp.tile([128, 4, NCLS], F32, tag="wfc")
        nc.sync.dma_start(wfc_sb[:], w_fc.rearrange("c p m -> p c m"))
        bfc_sb = wp.tile([NCLS, 1], F32, tag="bfc")
        nc.sync.dma_start(bfc_sb[:], b_fc.rearrange("c p -> p c"))

        # persistent conv weights for blocks 0..3 (L1, L2)
        wsb_persist = {}
        for i in range(4):
            for key in (f"b{i}w1", f"b{i}w2", f"b{i}wd"):
                if key not in wdram:
                    continue
                d = wdram[key]
                if key in PACKED:
                    t = wp.tile([128, 6, d.shape[2]], DT, tag=f"w_{key}")
                    nc.sync.dma_start(t[:], d.rearrange("g p m -> p g m"))
                    wsb_persist[key] = t
                else:
                    nchin, cin_p, KK, co = d.shape
                    ts = []
                    for ci in range(nchin):
                        t = wp.tile([cin_p, KK, co], DT, tag=f"w_{key}_{ci}")
                        nc.sync.dma_start(t[:], d[ci])
                        ts.append(t)
                    wsb_persist[key] = ts

        A1 = tiles[(0, "A")]

        def dup_dma(dst, a, b):
            """maintain dst[64:128, j] = dst[0:64, j+1] over x-span [a, b)."""
            nc.sync.dma_start(dst[64:128, a - 1:b - 1], dst[0:64, a:b])

        # ---- stem + maxpool: 6 strips x 32 stem rows, chunk-pairs col-packed ----
        with tc.tile_pool(name="stems", bufs=2) as stems, \
             tc.tile_pool(name="stemt", bufs=2) as stemt, \
             tc.tile_pool(name="imcp", bufs=6) as imcp:
            prev_strip = None
            for s in range(6):
                strip = stems.tile([64, 6336], DT, tag="strip")
                if s > 0:
                    nc.vector.tensor_copy(strip[:, 0:192], prev_strip[:, 6144:6336])
                c0 = s * 6144
                for kp in range(6):
                    cA = c0 + 2 * kp * 512
                    cB = cA + 512
                    tA = imcp.tile([128, 512], DT, tag="iA")
                    tB = imcp.tile([128, 512], DT, tag="iB")
                    tR = imcp.tile([64, 512], DT, tag="iR")
                    nc.sync.dma_start(tA[:], imc[0:128, cA:cA + 512])
                    nc.sync.dma_start(tB[:], imc[0:128, cB:cB + 512])
                    nc.sync.dma_start(tR[0:32, :], imc[128:160, cA:cA + 512])
                    nc.sync.dma_start(tR[32:64, :], imc[128:160, cB:cB + 512])
                    ps = psum.tile([128, 512], F32, tag="ps", name="ps_stem")
                    nc.tensor.matmul(ps[0:64, :], ws1[:], tA[:], start=True, stop=False,
                                     tile_position=(0, 0), skip_group_check=True)
                    nc.tensor.matmul(ps[64:128, :], ws1[:], tB[:], start=True, stop=False,
                                     tile_position=(0, 64), skip_group_check=True)
                    nc.tensor.matmul(ps[0:64, :], ws2t[0:32, :], tR[0:32, :],
                                     start=False, stop=True,
                                     tile_position=(0, 0), skip_group_check=True)
                    nc.tensor.matmul(ps[64:128, :], ws2t[32:64, :], tR[32:64, :],
                                     start=False, stop=True,
                                     tile_position=(32, 64), skip_group_check=True)
                    nc.scalar.activation(strip[:, 192 + 2 * kp * 512:192 + (2 * kp + 1) * 512],
                                         ps[0:64, :], AF.Relu, bias=bstem_sb[:, 0:1])
                    nc.scalar.activation(strip[:, 192 + (2 * kp + 1) * 512:192 + (2 * kp + 2) * 512],
                                         ps[64:128, :], AF.Relu, bias=bstem_sb[:, 0:1])
                # maxpool 16 output rows
                st3 = strip.rearrange("p (h w) -> p h w", w=192)
                t = stemt.tile([64, 16, 192], DT, tag="pt")
                nc.vector.tensor_max(t[:], st3[:, 1:33:2, :], st3[:, 2:33:2, :])
                if s == 0:
                    nc.vector.tensor_max(t[:, 1:16, :], t[:, 1:16, :], st3[:, 2:31:2, :])
                else:
                    nc.vector.tensor_max(t[:], t[:], st3[:, 0:31:2, :])
                Ev = t[:, :, 0:191:2]
                Ov = t[:, :, 1:192:2]
                dstv = A1.rearrange("p c (h w) -> p c h w", w=98)[:, 0]
                R0 = 16 * s + 1
                nc.vector.tensor_max(dstv[0:64, R0:R0 + 16, 1:97], Ev, Ov)
                nc.vector.tensor_max(dstv[0:64, R0:R0 + 16, 2:97],
                                     dstv[0:64, R0:R0 + 16, 2:97], Ov[:, :, 0:95])
                dup_dma(A1[:, 0], R0 * 98, (R0 + 16) * 98)
                prev_strip = strip

        if "pool" in dbg:
            nc.sync.dma_start(dbg["pool"][:], A1[0:64].rearrange("p c f -> p (c f)"))

        # ---- conv emission ----
        wstream = stack.enter_context(tc.tile_pool(name="wstream", bufs=8))
        wsb_stream = {}

        def get_wsb(key):
            if key in wsb_persist:
                return wsb_persist[key]
            if key not in wsb_stream:
                d = wdram[key]
                nchin, cin_p, KK, co = d.shape
                ts = []
                for ci in range(nchin):
                    t = wstream.tile([cin_p, KK, co], DT, tag="wstream")
                    nc.sync.dma_start(t[:], d[ci])
                    ts.append(t)
                wsb_stream[key] = ts
            return wsb_stream[key]

        def combine(mode, i, bias, co, ps_half, NA, wout, dstv, resv, r0, nr):
            """write one spatial chunk from a psum region."""
            ps3 = ps_half.rearrange("p (a b) -> p a b", b=wout)
            dchunk = dstv[0:64, r0 + 1:r0 + 1 + nr, 1:1 + wout] if dstv.shape[0] == 128 \
                else dstv[:, r0 + 1:r0 + 1 + nr, 1:1 + wout]
            if mode == "res":
                rchunk = resv[0:64, r0 + 1:r0 + 1 + nr, 1:1 + wout]
                nc.vector.tensor_add(dchunk, ps3, rchunk)
                nc.scalar.activation(dchunk, dchunk, AF.Relu, bias=bias[:, co:co + 1])
            else:
                nc.scalar.activation(dchunk, ps3, AF.Relu, bias=bias[:, co:co + 1])

        def emit_conv_packed_l1(src, dst, i, wkey, mode, res_src=None):
            """L1 convs (cin=cout=64, stride1): offset-pair K-packing plus
            2-spatial-chunk col-packing.  src/dst are [128, 1, 98*98] with dup."""
            wpt = get_wsb(wkey)           # [128, 6, 64]
            bias = bias_sb[f"b{i}comb_b" if wkey.endswith("w2") else f"b{i}w1_b"]
            srcv = src.rearrange("p c (h w) -> p c h w", w=98)[:, 0]
            dstv = dst.rearrange("p c (h w) -> p c h w", w=98)[:, 0]
            resv = res_src.rearrange("p c (h w) -> p c h w", w=98)[:, 0] \
                if res_src is not None else None
            chunks = row_chunks(96, 96)
            assert len(chunks) % 2 == 0
            for pi in range(0, len(chunks), 2):
                (rA, nA), (rB, nB) = chunks[pi], chunks[pi + 1]
                NA, NB = nA * 96, nB * 96
                ps = psum.tile([128, 512], F32, tag="ps", name="ps_l1")
                for g in range(3):
                    nc.tensor.matmul(ps[0:64, :NA], wpt[:, g, :],
                                     srcv[:, rA + g:rA + g + nA, 0:96],
                                     start=(g == 0), stop=False,
                                     tile_position=(0, 0), skip_group_check=True)
                    nc.tensor.matmul(ps[64:128, :NB], wpt[:, g, :],
                                     srcv[:, rB + g:rB + g + nB, 0:96],
                                     start=(g == 0), stop=False,
                                     tile_position=(0, 64), skip_group_check=True)
                for g in range(3):
                    last = (g == 2)
                    nc.tensor.matmul(ps[0:64, :NA], wpt[0:64, 3 + g, :],
                                     srcv[0:64, rA + g:rA + g + nA, 2:98],
                                     start=False, stop=last,
                                     tile_position=(0, 0), skip_group_check=True)
                    nc.tensor.matmul(ps[64:128, :NB], wpt[64:128, 3 + g, :],
                                     srcv[64:128, rB + g:rB + g + nB, 1:97],
                                     start=False, stop=last,
                                     tile_position=(64, 64), skip_group_check=True)
                combine(mode, i, bias, 0, ps[0:64, :NA], NA, 96, dstv, resv, rA, nA)
                combine(mode, i, bias, 0, ps[64:128, :NB], NB, 96, dstv, resv, rB, nB)
                dup_dma(dst[:, 0], (rA + 1) * 98, (rA + 1 + nA) * 98)
                dup_dma(dst[:, 0], (rB + 1) * 98, (rB + 1 + nB) * 98)

        def emit_conv_packed_b2c1(src, dst):
            """b2 conv1: 64->128 stride 2 from l1A (dup) into l2T."""
            wpt = get_wsb("b2w1")         # [128, 6, 128]
            bias = bias_sb["b2w1_b"]
            srcv = src.rearrange("p c (h w) -> p c h w", w=98)[:, 0]
            dstv = dst.rearrange("p c (h w) -> p c h w", w=50)[:, 0]
            for (r0, nr) in row_chunks(48, 48):
                N = nr * 48
                ps = psum.tile([128, 512], F32, tag="ps", name="ps_b2c1")[:, :N]
                for g in range(3):
                    rhs = srcv[:, 2 * r0 + g:2 * r0 + g + 2 * nr - 1:2, 0:95:2]
                    nc.tensor.matmul(ps, wpt[:, g, :], rhs,
                                     start=(g == 0), stop=False, skip_group_check=True)
                for g in range(3):
                    rhs = srcv[0:64, 2 * r0 + g:2 * r0 + g + 2 * nr - 1:2, 2:97:2]
                    nc.tensor.matmul(ps, wpt[0:64, 3 + g, :], rhs,
                                     start=False, stop=(g == 2), skip_group_check=True)
                ps3 = ps.rearrange("p (a b) -> p a b", b=48)
                nc.scalar.activation(dstv[:, r0 + 1:r0 + 1 + nr, 1:49], ps3,
                                     AF.Relu, bias=bias[:, 0:1])

        def conv_rhs(srcv, ci, wout, stride, r0, nr, ky, kx, src128):
            sl = srcv[0:64, ci] if src128 else srcv[:, ci]
            if stride == 1:
                return sl[:, r0 + ky:r0 + ky + nr, kx:kx + wout]
            return sl[:, 2 * r0 + ky:2 * r0 + ky + 2 * nr - 1:2,
                      kx:kx + 2 * wout - 1:2]

        def emit_conv(src, dst, i, wkey, mode, res_src=None):
            ci_, co_, st_ = BLOCK_SPECS[i]
            if wkey.endswith("w2"):
                cin, cout, stride = co_, co_, 1
                hin = hout = 96 // (2 ** (i // 2))
                bias = bias_sb[f"b{i}comb_b"]
            else:
                cin, cout, stride = ci_, co_, st_
                hout = 96 // (2 ** (i // 2))
                hin = hout * stride
                bias = bias_sb[f"b{i}w1_b"]
            wout = hout
            cin_p, nchin = cp_of(cin), nch_of(cin)
            cop, ncho = cp_of(cout), nch_of(cout)
            src128 = (src.shape[0] == 128 and cin == 64)
            srcv = src.rearrange("p c (h w) -> p c h w", w=hin + 2)
            dstv = dst.rearrange("p c (h w) -> p c h w", w=hout + 2)
            wts = get_wsb(wkey)
            if mode == "res_ds":
                wdts = get_wsb(f"b{i}wd")
                nch_ds = nch_of(ci_)
                resv = res_src.rearrange("p c (h w) -> p c h w", w=hout * st_ + 2)
                res128 = (res_src.shape[0] == 128 and ci_ == 64)
            if mode == "res":
                resv = res_src.rearrange("p c (h w) -> p c h w", w=hout + 2)
            offs = [(a, b) for a in range(3) for b in range(3)]
            for co in range(ncho):
                co_sl = slice(co * 128, co * 128 + cop)
                for (r0, nr) in row_chunks(hout, wout):
                    N = nr * wout
                    ps = psum.tile([128, 512], F32, tag="ps", name="ps_conv")[:cop, :N]
                    ps3 = ps.rearrange("p (a b) -> p a b", b=wout)
                    n_mm = nchin * 9 + (nch_ds if mode == "res_ds" else 0)
                    idx = 0
                    for ci in range(nchin):
                        for k, (ky, kx) in enumerate(offs):
                            rhs = conv_rhs(srcv, ci, wout, stride, r0, nr, ky, kx, src128)
                            nc.tensor.matmul(ps, wts[ci][:, k, co_sl], rhs,
                                             start=(idx == 0), stop=(idx == n_mm - 1))
                            idx += 1
                    if mode == "res_ds":
                        for ci in range(nch_ds):
                            sl = resv[0:64, ci] if res128 else resv[:, ci]
                            rhs = sl[:, 2 * r0 + 1:2 * r0 + 1 + 2 * nr - 1:2,
                                     1:1 + 2 * wout - 1:2]
                            nc.tensor.matmul(ps, wdts[ci][:, 0, co_sl], rhs,
                                             start=False, stop=(idx == n_mm - 1))
                            idx += 1
                    dchunk = dstv[:, co, r0 + 1:r0 + 1 + nr, 1:1 + wout]
                    if mode == "res":
                        rchunk = resv[:, co, r0 + 1:r0 + 1 + nr, 1:1 + wout]
                        nc.vector.tensor_add(dchunk, ps3, rchunk)
                        nc.scalar.activation(dchunk, dchunk, AF.Relu,
                                             bias=bias[:, co:co + 1])
                    else:
                        nc.scalar.activation(dchunk, ps3, AF.Relu,
                                             bias=bias[:, co:co + 1])

        # ---- block sequence ----
        T1, B1 = tiles[(0, "T")], tiles[(0, "B")]
        emit_conv_packed_l1(A1, T1, 0, "b0w1", "relu")
        emit_conv_packed_l1(T1, B1, 0, "b0w2", "res", res_src=A1)
        if "b0" in dbg:
            nc.sync.dma_start(dbg["b0"][:], B1[0:64].rearrange("p c f -> p (c f)"))
        emit_conv_packed_l1(B1, T1, 1, "b1w1", "relu")
        emit_conv_packed_l1(T1, A1, 1, "b1w2", "res", res_src=B1)
        if "b1" in dbg:
            nc.sync.dma_start(dbg["b1"][:], A1[0:64].rearrange("p c f -> p (c f)"))

        # L2 first block: packed conv1, then conv2 + ds accumulate
        T2, B2, F2 = tiles[(1, "T")], tiles[(1, "B")], tiles[(1, "A")]
        emit_conv_packed_b2c1(A1, T2)
        emit_conv(T2, B2, 2, "b2w2", "res_ds", res_src=A1)
        if "b2" in dbg:
            nc.sync.dma_start(dbg["b2"][:], B2.rearrange("p c f -> p (c f)"))
        emit_conv(B2, T2, 3, "b3w1", "relu")
        emit_conv(T2, F2, 3, "b3w2", "res", res_src=B2)
        if "b3" in dbg:
            nc.sync.dma_start(dbg["b3"][:], F2.rearrange("p c f -> p (c f)"))

        prevF = F2
        for li in (2, 3):
            T, Bt, F = tiles[(li, "T")], tiles[(li, "B")], tiles[(li, "A")]
            i0, i1 = 2 * li, 2 * li + 1
            emit_conv(prevF, T, i0, f"b{i0}w1", "relu")
            emit_conv(T, Bt, i0, f"b{i0}w2", "res_ds", res_src=prevF)
            if f"b{i0}" in dbg:
                nc.sync.dma_start(dbg[f"b{i0}"][:], Bt.rearrange("p c f -> p (c f)"))
            emit_conv(Bt, T, i1, f"b{i1}w1", "relu")
            emit_conv(T, F, i1, f"b{i1}w2", "res", res_src=Bt)
            if f"b{i1}" in dbg:
                nc.sync.dma_start(dbg[f"b{i1}"][:], F.rearrange("p c f -> p (c f)"))
            prevF = F

        # ---- global avg pool + fc (fp32) ----
        F4 = prevF
        f4v = F4.rearrange("p c (h w) -> p c h w", w=14)
        h = acts.tile([128, 4], F32, tag="h")
        for c in range(4):
            nc.vector.reduce_sum(h[:, c:c + 1], f4v[:, c, 1:13, 1:13],
                                 axis=mybir.AxisListType.XY)
        h1 = acts.tile([128, 4], F32, tag="h1")
        nc.scalar.activation(h1[:], h[:], AF.Copy, scale=float(1.0 / 144.0))
        pf = psum.tile([128, 512], F32, tag="ps", name="ps_fc")[:NCLS, :1]
        for c in range(4):
            nc.tensor.matmul(pf, wfc_sb[:, c, :], h1[:, c:c + 1],
                             start=(c == 0), stop=(c == 3))
        osb = acts.tile([NCLS, 1], F32, tag="osb")
        nc.scalar.activation(osb[:], pf, AF.Identity, bias=bfc_sb[:, 0:1])
        nc.sync.dma_start(out_d[:], osb[:])

    split_waits(nc, mybir)
    return nc


# ---------------------------------------------------------------------------
# entry point
# ---------------------------------------------------------------------------

_CACHE = {}


def _get_nc(debug_taps=()):
    key = ("nc", tuple(debug_taps))
    if key not in _CACHE:
        _CACHE[key] = build_nc(debug_taps)
    return _CACHE[key]


def run_cores(in_maps, debug_taps=(), trace=False, tmpdir=None):
    from concourse.bass_utils import run_bass_kernel_spmd
    nc = _get_nc(debug_taps)
    return run_bass_kernel_spmd(nc, in_maps, list(range(NB)), trace=trace,
                                tmpdir=tmpdir)


def kernel(x, w_gate, params):
    x = np.asarray(x, np.float32)
    idx, g = host_gating(x, w_gate)
    experts = {}
    for e in set(idx.tolist()):
        experts[e] = prep_expert(params, e)
    in_maps = []
    for b in range(NB):
        m = dict(experts[idx[b]])
        m["imc"] = im2col_stem(x[b])
        in_maps.append(m)
    res = run_cores(in_maps)
    out = np.stack([res.results[b]["out"][:, 0] for b in range(NB)], axis=0)
    return (out * g[:, None]).astype(np.float32)
